# revision 14
# baseline (speedup 1.0000x reference)
"""Trainium2 Bass kernel for nn_DecoderModel_73358041415847 (moe_routing).

Strategy (expert-parallel, host-dispatched):
  - dataset_name assigns each graph (and, via batch, each node) to one of
    B=4 branches.  The host routes graphs+nodes to 8 shards: branch b goes
    to cores {2b, 2b+1}, split into two node-balanced halves.  Each core
    then runs a *dense* pipeline for its single branch — no on-device
    gather/select at all, and only the selected branch's FLOPs are spent
    (4x saving vs the reference's dense-MoE form).
  - Graph mean-pooling is a matmul against a host-built [M, G] one-hot
    matrix pre-scaled by 1/count, so pooled means come straight out of PSUM.
    Pooling matmuls for a graph-chunk are only emitted for node blocks that
    can actually contain that chunk's nodes (computed from the data).
  - Node MLP needs x with the feature dim on partitions; x tiles are
    transposed on-device via identity matmuls (fp32r, full PE rate).
  - All matmuls run as float32r (FP22 mantissa, 1-pass PE).

Outputs are returned as (graph_head, node_head, graph_var, node_var),
matching reference().
"""

import os

import numpy as np

D = 512        # node feature dim
HS = 768       # graph shared hidden
B = 4          # branches
GHD = 64       # graph head dim
OG = 2 * GHD   # 128
ON = 6         # 2 * NHD
NHD = 3
P = 128
NCORES = 8
JBLK = 512     # nodes per device-side block

_CACHE = {}


def _round_up(v, m):
    return ((int(v) + m - 1) // m) * m


def _g_chunks(G_pad, c0=None):
    if c0 is None or c0 >= G_pad:
        out = []
        off = 0
        while off < G_pad:
            sz = min(P, G_pad - off)
            out.append((off, sz))
            off += sz
        return out
    assert 0 < c0 <= P and G_pad - c0 <= P
    return [(0, c0), (c0, G_pad - c0)]


def _plan_shards(batch, dataset_name):
    """Partition graphs (and their contiguous node ranges) into 8 shards:
    2 node-balanced shards per branch."""
    G = int(dataset_name.shape[0])
    counts = np.bincount(batch, minlength=G).astype(np.int64)
    starts = np.zeros(G + 1, np.int64)
    starts[1:] = np.cumsum(counts)
    shards = []
    for b in range(B):
        gb = np.where(dataset_name == b)[0]
        nb = counts[gb]
        tot = int(nb.sum())
        if len(gb):
            c = np.cumsum(nb)
            k = int(np.searchsorted(c, tot / 2.0))
        else:
            k = 0
        for part in (gb[:k], gb[k:]):
            if len(part):
                node_idx = np.concatenate(
                    [np.arange(starts[g], starts[g + 1]) for g in part]
                )
            else:
                node_idx = np.zeros((0,), np.int64)
            shards.append((part, node_idx))
    return shards, counts


def _build_program(M_pad, G_pad, c0, pool_act):
    """Emit + compile the SPMD Bass program for one core. Cached.

    pool_act: per graph-chunk (first_block, last_block) range in which any
    core's nodes for that chunk can appear — pooling matmuls are only
    emitted there.
    """
    key = (M_pad, G_pad, c0, pool_act)
    if key in _CACHE:
        return _CACHE[key]

    from contextlib import ExitStack

    import concourse.tile as tile
    from concourse import bacc, mybir

    f32 = mybir.dt.float32
    f32r = mybir.dt.float32r
    AF = mybir.ActivationFunctionType
    use_tt = bool(int(os.environ.get("BASS_TT", "0")))

    NB = M_pad // JBLK
    GC = _g_chunks(G_pad, c0)
    assert len(pool_act) == len(GC)

    nc = bacc.Bacc()
    x_d = nc.declare_dram_parameter("x", [M_pad, D], f32r, isOutput=False)
    pt_d = nc.declare_dram_parameter("pt", [M_pad, G_pad], f32r, isOutput=False)
    wn1_d = nc.declare_dram_parameter("wn1", [D, D], f32r, isOutput=False)
    bn1_d = nc.declare_dram_parameter("bn1", [D], f32, isOutput=False)
    # wn2 padded to 8 cols on host (cols 0..5 real, 6..7 zero)
    wn2_d = nc.declare_dram_parameter("wn2", [D, 8], f32r, isOutput=False)
    bn2_d = nc.declare_dram_parameter("bn2", [ON], f32, isOutput=False)
    wg_d = nc.declare_dram_parameter("wg", [D, HS], f32r, isOutput=False)
    bg_d = nc.declare_dram_parameter("bg", [HS], f32, isOutput=False)
    wgh_d = nc.declare_dram_parameter("wgh", [HS, OG], f32r, isOutput=False)
    id_d = nc.declare_dram_parameter("ident", [P, P], f32r, isOutput=False)
    bgh_d = nc.declare_dram_parameter("bgh", [OG], f32, isOutput=False)
    on_out_d = nc.declare_dram_parameter("on_out", [ON, M_pad], f32, isOutput=True)
    og_out_d = nc.declare_dram_parameter("og_out", [OG, G_pad], f32, isOutput=True)

    with tile.TileContext(nc) as tc, ExitStack() as ctx:
        const = ctx.enter_context(tc.tile_pool(name="const", bufs=1))
        xpool = ctx.enter_context(tc.tile_pool(name="xp", bufs=3))
        ptpool = ctx.enter_context(tc.tile_pool(name="ptp", bufs=3))
        xtpool = ctx.enter_context(tc.tile_pool(name="xtp", bufs=2))
        hnpool = ctx.enter_context(tc.tile_pool(name="hnp", bufs=2))
        onpool = ctx.enter_context(tc.tile_pool(name="onp", bufs=3))
        ps_xg = ctx.enter_context(tc.tile_pool(name="ps_xg", bufs=1, space="PSUM"))
        ps_t = ctx.enter_context(tc.tile_pool(name="ps_t", bufs=2, space="PSUM"))
        ps_hn = ctx.enter_context(tc.tile_pool(name="ps_hn", bufs=2, space="PSUM"))
        ps_on = ctx.enter_context(tc.tile_pool(name="ps_on", bufs=2, space="PSUM"))

        # identity first (tiny, needed by the first transpose); node-phase
        # weights go on the gpsimd DMA queue so their triggers don't delay
        # the block-0 x/pt loads on the sync queue.
        ident_r = const.tile([P, P], f32r, name="ident_r")
        nc.sync.dma_start(ident_r, id_d[:])

        wn1_sb = const.tile([P, 4, D], f32r)
        nc.gpsimd.dma_start(wn1_sb, wn1_d[:].rearrange("(c p) n -> p c n", p=P))
        wn2_sb = const.tile([P, 4, 8], f32r)
        nc.gpsimd.dma_start(wn2_sb, wn2_d[:].rearrange("(c p) n -> p c n", p=P))
        bn1_sb = const.tile([P, 4], f32)
        nc.gpsimd.dma_start(bn1_sb, bn1_d[:].rearrange("(c p) -> p c", p=P))
        bn2_sb = const.tile([ON, 1], f32)
        nc.gpsimd.dma_start(bn2_sb, bn2_d[:].unsqueeze(1))

        # graph-phase weights: emitted after block 0 so they stay off the
        # critical lead-in path (they're only consumed at the very end).
        wg_sb = const.tile([P, 4, HS], f32r)
        wgh_sb = const.tile([P, 6, OG], f32r)
        bg_sb = const.tile([P, 6], f32)
        bgh_sb = const.tile([P, 1], f32)

        # persistent PSUM accumulators for the pooled graph means
        xg_ps = [
            ps_xg.tile([P, D], f32, tag=f"xg{gi}", name=f"xg{gi}")
            for gi in range(len(GC))
        ]

        xg_sb = const.tile([P, len(GC), D], f32r)
        xgt_sb = const.tile([P, 4, G_pad], f32r)
        hgt_sb = const.tile([P, 6, G_pad], f32r)
        og_sb = const.tile([OG, G_pad], f32)

        def emit_graph_chunk(gi):
            """Graph head for chunk gi: runs as soon as that chunk's pooling
            accumulation is complete, overlapping later node blocks."""
            goff, gsz = GC[gi]
            nc.vector.tensor_copy(out=xg_sb[:gsz, gi, :], in_=xg_ps[gi][:gsz, :])
            for dc in range(4):
                t_ps2 = ps_t.tile([P, JBLK], f32, tag="t", name="t_ps2")
                nc.tensor.matmul(
                    t_ps2[:, :gsz],
                    lhsT=xg_sb[:gsz, gi, dc * P : (dc + 1) * P],
                    rhs=ident_r[:gsz, :gsz],
                    start=True,
                    stop=True,
                )
                nc.vector.tensor_copy(
                    out=xgt_sb[:, dc, goff : goff + gsz], in_=t_ps2[:, :gsz]
                )
            for hc in range(6):
                g_ps = ps_hn.tile([P, JBLK], f32, tag="hn", name="g_ps")
                for dc in range(4):
                    nc.tensor.matmul(
                        g_ps[:, :gsz],
                        lhsT=wg_sb[:, dc, hc * P : (hc + 1) * P],
                        rhs=xgt_sb[:, dc, goff : goff + gsz],
                        start=(dc == 0),
                        stop=(dc == 3),
                    )
                nc.scalar.activation(
                    out=hgt_sb[:, hc, goff : goff + gsz], in_=g_ps[:, :gsz],
                    func=AF.Relu, bias=bg_sb[:, hc : hc + 1],
                )
            og_ps = ps_on.tile([P, JBLK], f32, tag="on", name="og_ps")
            for hc in range(6):
                nc.tensor.matmul(
                    og_ps[:, :gsz],
                    lhsT=wgh_sb[:, hc, :],
                    rhs=hgt_sb[:, hc, goff : goff + gsz],
                    start=(hc == 0),
                    stop=(hc == 5),
                )
            nc.scalar.activation(
                out=og_sb[:GHD, goff : goff + gsz], in_=og_ps[:GHD, :gsz],
                func=AF.Identity, bias=bgh_sb[:GHD, :],
            )
            nc.scalar.activation(
                out=og_sb[GHD:OG, goff : goff + gsz], in_=og_ps[GHD:OG, :gsz],
                func=AF.Square, bias=bgh_sb[GHD:OG, :],
            )
            nc.sync.dma_start(
                og_out_d[:, goff : goff + gsz], og_sb[:, goff : goff + gsz]
            )

        for jb in range(NB):
            # per-a (128-row) loads: the first pooling matmul only needs
            # the a=0 slices, so the PE starts ~4x earlier on each block.
            x_nat = xpool.tile([P, 4, D], f32r, tag="x")
            pt_sb = ptpool.tile([P, 4, G_pad], f32r, tag="pt")
            for a in range(4):
                r0 = jb * JBLK + a * P
                nc.sync.dma_start(x_nat[:, a, :], x_d[r0 : r0 + P, :])
                nc.sync.dma_start(pt_sb[:, a, :], pt_d[r0 : r0 + P, :])

            # graph mean pooling accumulation: xg[g, :] += PT_chunk.T @ x
            for a in range(4):
                for gi, (goff, gsz) in enumerate(GC):
                    bf, bl = pool_act[gi]
                    if not (bf <= jb <= bl):
                        continue
                    nc.tensor.matmul(
                        xg_ps[gi][:gsz, :],
                        lhsT=pt_sb[:, a, goff : goff + gsz],
                        rhs=x_nat[:, a, :],
                        start=(jb == bf and a == 0),
                        stop=(jb == bl and a == 3),
                        skip_group_check=True,
                    )

            # transpose x block: xt[p_dlow, dc, j] = x[node j, dc*128+p]
            xt_sb = xtpool.tile([P, 4, JBLK], f32r, tag="xt")
            for dc in range(4):
                if use_tt:
                    t_ps = ps_t.tile([P, JBLK], f32r, tag="t")
                    for a in range(4):
                        nc.tensor.transpose(
                            t_ps[:, a * P : (a + 1) * P],
                            x_nat[:, a, dc * P : (dc + 1) * P],
                            ident_r,
                        )
                else:
                    t_ps = ps_t.tile([P, JBLK], f32, tag="t")
                    for a in range(4):
                        nc.tensor.matmul(
                            t_ps[:, a * P : (a + 1) * P],
                            lhsT=x_nat[:, a, dc * P : (dc + 1) * P],
                            rhs=ident_r,
                            start=True,
                            stop=True,
                        )
                nc.vector.tensor_copy(out=xt_sb[:, dc, :], in_=t_ps)

            # node MLP layer 1 (transposed): hn_T = relu(Wn1.T-chunks @ x_T + b)
            hn_sb = hnpool.tile([P, 4, JBLK], f32r, tag="hn")
            for d2c in range(4):
                h_ps = ps_hn.tile([P, JBLK], f32, tag="hn")
                for dc in range(4):
                    nc.tensor.matmul(
                        h_ps,
                        lhsT=wn1_sb[:, dc, d2c * P : (d2c + 1) * P],
                        rhs=xt_sb[:, dc, :],
                        start=(dc == 0),
                        stop=(dc == 3),
                    )
                nc.scalar.activation(
                    out=hn_sb[:, d2c, :],
                    in_=h_ps,
                    func=AF.Relu,
                    bias=bn1_sb[:, d2c : d2c + 1],
                )

            # node head: on_T[o, j] = Wn2.T @ hn_T.  Identity and Square
            # copies over all 6 rows; DMA (exempt from the 32-aligned
            # partition-start rule) picks head rows from the identity copy
            # and var rows from the squared copy.
            o_ps = ps_on.tile([P, JBLK], f32, tag="on")
            for d2c in range(4):
                nc.tensor.matmul(
                    o_ps[:ON, :],
                    lhsT=wn2_sb[:, d2c, 0:ON],
                    rhs=hn_sb[:, d2c, :],
                    start=(d2c == 0),
                    stop=(d2c == 3),
                )
            on_id = onpool.tile([ON, JBLK], f32, tag="on_id")
            on_sq = onpool.tile([ON, JBLK], f32, tag="on_sq")
            nc.scalar.activation(
                out=on_id, in_=o_ps[:ON, :], func=AF.Identity, bias=bn2_sb[:ON, :]
            )
            nc.scalar.activation(
                out=on_sq, in_=o_ps[:ON, :], func=AF.Square, bias=bn2_sb[:ON, :]
            )
            nc.sync.dma_start(
                on_out_d[:NHD, jb * JBLK : (jb + 1) * JBLK], on_id[:NHD, :]
            )
            nc.sync.dma_start(
                on_out_d[NHD:ON, jb * JBLK : (jb + 1) * JBLK], on_sq[NHD:ON, :]
            )

            if jb == 0:
                nc.gpsimd.dma_start(wg_sb, wg_d[:].rearrange("(c p) n -> p c n", p=P))
                nc.gpsimd.dma_start(wgh_sb, wgh_d[:].rearrange("(c p) n -> p c n", p=P))
                nc.gpsimd.dma_start(bg_sb, bg_d[:].rearrange("(c p) -> p c", p=P))
                nc.gpsimd.dma_start(bgh_sb, bgh_d[:].unsqueeze(1))

            for gi in range(len(GC)):
                if pool_act[gi][1] == jb:
                    emit_graph_chunk(gi)

        # (graph-head emission happens per-chunk inside the block loop)

    nc.compile()
    _CACHE[key] = nc
    return nc


def _pad_wn2(w):
    out = np.zeros((w.shape[0], 8), np.float32)
    out[:, 0:ON] = w
    return out


def _make_in_maps(x, batch, dataset_name, Wg_shared, bg_shared, Wg_head, bg_head,
                  Wn1, bn1, Wn2, bn2, shards, counts, M_pad, G_pad):
    in_maps = []
    for ci in range(NCORES):
        b = ci // 2
        graphs, node_idx = shards[ci]
        n = len(node_idx)
        g = len(graphs)
        x_sh = np.zeros((M_pad, D), np.float32)
        if n:
            x_sh[:n] = x[node_idx]
        pt = np.zeros((M_pad, G_pad), np.float32)
        if n:
            gcounts = counts[graphs]
            glocal = np.repeat(np.arange(g), gcounts)
            pt[np.arange(n), glocal] = (
                1.0 / np.maximum(gcounts, 1).astype(np.float32)
            )[glocal]
        in_maps.append({
            "x": x_sh,
            "ident": np.eye(P, dtype=np.float32),
            "pt": pt,
            "wn1": np.ascontiguousarray(Wn1[b]),
            "bn1": np.ascontiguousarray(bn1[b]),
            "wn2": _pad_wn2(Wn2[b]),
            "bn2": np.ascontiguousarray(bn2[b]),
            "wg": np.ascontiguousarray(Wg_shared[b]),
            "bg": np.ascontiguousarray(bg_shared[b]),
            "wgh": np.ascontiguousarray(Wg_head[b]),
            "bgh": np.ascontiguousarray(bg_head[b]),
        })
    return in_maps


def _choose_c0(shards, counts, M_pad, G_pad):
    """Pick the chunk-0/chunk-1 graph boundary so chunk 0's nodes end before
    the last node block on every core — lets chunk 0's graph head overlap
    the final node block.  Returns None when a single chunk suffices."""
    if G_pad <= P:
        return None
    NB = M_pad // JBLK
    lim = (NB - 1) * JBLK
    c0 = P
    for graphs, node_idx in shards:
        g = len(graphs)
        if g == 0:
            continue
        cs = np.cumsum(counts[graphs])
        pc = int(np.searchsorted(cs, lim, "right"))
        c0 = min(c0, pc)
    c0 = max(c0, G_pad - P, 1)
    return min(c0, P)


def _pool_activity(shards, counts, M_pad, GC):
    """Per graph-chunk, the (first, last) node-block range (over all cores)
    where that chunk's nodes can appear."""
    act = []
    for goff, gsz in GC:
        bf, bl = None, None
        for graphs, node_idx in shards:
            g = len(graphs)
            if g == 0 or goff >= g:
                continue
            gcounts = counts[graphs]
            glocal = np.repeat(np.arange(g), gcounts)
            jf = int(np.searchsorted(glocal, goff, "left"))
            jl = int(np.searchsorted(glocal, min(goff + gsz, g), "left")) - 1
            if jl < jf:
                continue
            cbf, cbl = jf // JBLK, jl // JBLK
            bf = cbf if bf is None else min(bf, cbf)
            bl = cbl if bl is None else max(bl, cbl)
        if bf is None:
            bf, bl = 0, 0  # chunk empty everywhere: one start-MM writes zeros
        act.append((bf, bl))
    return tuple(act)


def _run(inputs, trace=False, trace_kwargs=None):
    from concourse.bass_utils import run_bass_kernel_spmd

    x = np.asarray(inputs["x"], np.float32)
    batch = np.asarray(inputs["batch"])
    dataset_name = np.asarray(inputs["dataset_name"])
    N = x.shape[0]
    G = dataset_name.shape[0]

    shards, counts = _plan_shards(batch, dataset_name)
    M_pad = max(JBLK, _round_up(max(len(s[1]) for s in shards), JBLK))
    G_pad = max(16, _round_up(max(len(s[0]) for s in shards), 16))
    c0 = _choose_c0(shards, counts, M_pad, G_pad)
    GC = _g_chunks(G_pad, c0)
    pool_act = _pool_activity(shards, counts, M_pad, GC)

    nc = _build_program(M_pad, G_pad, c0, pool_act)
    in_maps = _make_in_maps(
        x, batch, dataset_name,
        np.asarray(inputs["Wg_shared"], np.float32),
        np.asarray(inputs["bg_shared"], np.float32),
        np.asarray(inputs["Wg_head"], np.float32),
        np.asarray(inputs["bg_head"], np.float32),
        np.asarray(inputs["Wn1"], np.float32),
        np.asarray(inputs["bn1"], np.float32),
        np.asarray(inputs["Wn2"], np.float32),
        np.asarray(inputs["bn2"], np.float32),
        shards, counts, M_pad, G_pad,
    )
    kw = {}
    if trace:
        kw["trace"] = True
        if trace_kwargs:
            kw.update(trace_kwargs)
    res = run_bass_kernel_spmd(nc, in_maps, core_ids=list(range(NCORES)), **kw)

    graph_head = np.zeros((G, GHD), np.float32)
    graph_var = np.zeros((G, GHD), np.float32)
    node_head = np.zeros((N, NHD), np.float32)
    node_var = np.zeros((N, NHD), np.float32)
    for ci in range(NCORES):
        graphs, node_idx = shards[ci]
        g, n = len(graphs), len(node_idx)
        og = res.results[ci]["og_out"]       # [128, G_pad]
        on = res.results[ci]["on_out"]       # [6, M_pad]
        if g:
            graph_head[graphs] = og[:GHD, :g].T
            graph_var[graphs] = og[GHD:OG, :g].T
        if n:
            node_head[node_idx] = on[:NHD, :n].T
            node_var[node_idx] = on[NHD:ON, :n].T
    return (graph_head, node_head, graph_var, node_var), res


def kernel(**inputs):
    outs, _ = _run(inputs, trace=False)
    return outs


# revision 15
# speedup vs baseline: 1.1039x; 1.1039x over previous
"""Trainium2 Bass kernel for nn_DecoderModel_73358041415847 (moe_routing).

Strategy (expert-parallel, host-dispatched):
  - dataset_name assigns each graph (and, via batch, each node) to one of
    B=4 branches.  The host routes graphs+nodes to 8 shards: branch b goes
    to cores {2b, 2b+1}, split into two node-balanced halves.  Each core
    then runs a *dense* pipeline for its single branch — no on-device
    gather/select at all, and only the selected branch's FLOPs are spent
    (4x saving vs the reference's dense-MoE form).
  - Graph mean-pooling is a matmul against a host-built [M, G] one-hot
    matrix pre-scaled by 1/count, so pooled means come straight out of PSUM.
    Pooling matmuls for a graph-chunk are only emitted for node blocks that
    can actually contain that chunk's nodes (computed from the data).
  - Node MLP needs x with the feature dim on partitions; x tiles are
    transposed on-device via identity matmuls (fp32r, full PE rate).
  - All matmuls run as float32r (FP22 mantissa, 1-pass PE).

Outputs are returned as (graph_head, node_head, graph_var, node_var),
matching reference().
"""

import os

import numpy as np

D = 512        # node feature dim
HS = 768       # graph shared hidden
B = 4          # branches
GHD = 64       # graph head dim
OG = 2 * GHD   # 128
ON = 6         # 2 * NHD
NHD = 3
P = 128
NCORES = 8
JBLK = 512     # nodes per device-side block

_CACHE = {}


def _round_up(v, m):
    return ((int(v) + m - 1) // m) * m


def _g_chunks(G_pad, c0=None):
    if c0 is None or c0 >= G_pad:
        out = []
        off = 0
        while off < G_pad:
            sz = min(P, G_pad - off)
            out.append((off, sz))
            off += sz
        return out
    assert 0 < c0 <= P and G_pad - c0 <= P
    return [(0, c0), (c0, G_pad - c0)]


def _plan_shards(batch, dataset_name):
    """Partition graphs (and their contiguous node ranges) into 8 shards:
    2 node-balanced shards per branch."""
    G = int(dataset_name.shape[0])
    counts = np.bincount(batch, minlength=G).astype(np.int64)
    starts = np.zeros(G + 1, np.int64)
    starts[1:] = np.cumsum(counts)
    shards = []
    for b in range(B):
        gb = np.where(dataset_name == b)[0]
        nb = counts[gb]
        tot = int(nb.sum())
        if len(gb):
            c = np.cumsum(nb)
            k = int(np.searchsorted(c, tot / 2.0))
        else:
            k = 0
        for part in (gb[:k], gb[k:]):
            if len(part):
                node_idx = np.concatenate(
                    [np.arange(starts[g], starts[g + 1]) for g in part]
                )
            else:
                node_idx = np.zeros((0,), np.int64)
            shards.append((part, node_idx))
    return shards, counts


def _build_program(M_pad, G_pad, c0, pool_act):
    """Emit + compile the SPMD Bass program for one core. Cached.

    pool_act: per graph-chunk (first_block, last_block) range in which any
    core's nodes for that chunk can appear — pooling matmuls are only
    emitted there.
    """
    key = (M_pad, G_pad, c0, pool_act)
    if key in _CACHE:
        return _CACHE[key]

    from contextlib import ExitStack

    import concourse.tile as tile
    from concourse import bacc, mybir

    f32 = mybir.dt.float32
    f32r = mybir.dt.float32r
    AF = mybir.ActivationFunctionType
    use_tt = bool(int(os.environ.get("BASS_TT", "0")))

    NB = M_pad // JBLK
    GC = _g_chunks(G_pad, c0)
    assert len(pool_act) == len(GC)

    nc = bacc.Bacc()
    x_d = nc.declare_dram_parameter("x", [M_pad, D], f32r, isOutput=False)
    pt_d = nc.declare_dram_parameter("pt", [M_pad, G_pad], f32r, isOutput=False)
    wn1_d = nc.declare_dram_parameter("wn1", [D, D], f32r, isOutput=False)
    bn1_d = nc.declare_dram_parameter("bn1", [D], f32, isOutput=False)
    # wn2 padded to 8 cols on host (cols 0..5 real, 6..7 zero)
    wn2_d = nc.declare_dram_parameter("wn2", [D, 8], f32r, isOutput=False)
    bn2_d = nc.declare_dram_parameter("bn2", [ON], f32, isOutput=False)
    wg_d = nc.declare_dram_parameter("wg", [D, HS], f32r, isOutput=False)
    bg_d = nc.declare_dram_parameter("bg", [HS], f32, isOutput=False)
    wgh_d = nc.declare_dram_parameter("wgh", [HS, OG], f32r, isOutput=False)
    id_d = nc.declare_dram_parameter("ident", [P, P], f32r, isOutput=False)
    bgh_d = nc.declare_dram_parameter("bgh", [OG], f32, isOutput=False)
    on_out_d = nc.declare_dram_parameter("on_out", [ON, M_pad], f32, isOutput=True)
    og_out_d = nc.declare_dram_parameter("og_out", [OG, G_pad], f32, isOutput=True)

    with tile.TileContext(nc) as tc, ExitStack() as ctx:
        const = ctx.enter_context(tc.tile_pool(name="const", bufs=1))
        xpool = ctx.enter_context(tc.tile_pool(name="xp", bufs=3))
        ptpool = ctx.enter_context(tc.tile_pool(name="ptp", bufs=3))
        xtpool = ctx.enter_context(tc.tile_pool(name="xtp", bufs=2))
        hnpool = ctx.enter_context(tc.tile_pool(name="hnp", bufs=2))
        onpool = ctx.enter_context(tc.tile_pool(name="onp", bufs=3))
        ps_xg = ctx.enter_context(tc.tile_pool(name="ps_xg", bufs=1, space="PSUM"))
        ps_t = ctx.enter_context(tc.tile_pool(name="ps_t", bufs=2, space="PSUM"))
        ps_hn = ctx.enter_context(tc.tile_pool(name="ps_hn", bufs=2, space="PSUM"))
        ps_on = ctx.enter_context(tc.tile_pool(name="ps_on", bufs=2, space="PSUM"))

        # identity first (tiny, needed by the first transpose); node-phase
        # weights go on the gpsimd DMA queue so their triggers don't delay
        # the block-0 x/pt loads on the sync queue.
        ident_r = const.tile([P, P], f32r, name="ident_r")
        nc.sync.dma_start(ident_r, id_d[:])

        wn1_sb = const.tile([P, 4, D], f32r)
        nc.gpsimd.dma_start(wn1_sb, wn1_d[:].rearrange("(c p) n -> p c n", p=P))
        wn2_sb = const.tile([P, 4, 8], f32r)
        nc.gpsimd.dma_start(wn2_sb, wn2_d[:].rearrange("(c p) n -> p c n", p=P))
        bn1_sb = const.tile([P, 4], f32)
        nc.gpsimd.dma_start(bn1_sb, bn1_d[:].rearrange("(c p) -> p c", p=P))
        bn2_sb = const.tile([ON, 1], f32)
        nc.gpsimd.dma_start(bn2_sb, bn2_d[:].unsqueeze(1))

        # graph-phase weights: emitted after block 0 so they stay off the
        # critical lead-in path (they're only consumed at the very end).
        wg_sb = const.tile([P, 4, HS], f32r)
        wgh_sb = const.tile([P, 6, OG], f32r)
        bg_sb = const.tile([P, 6], f32)
        bgh_sb = const.tile([P, 1], f32)

        # persistent PSUM accumulators for the pooled graph means
        xg_ps = [
            ps_xg.tile([P, D], f32, tag=f"xg{gi}", name=f"xg{gi}")
            for gi in range(len(GC))
        ]

        xg_sb = const.tile([P, len(GC), D], f32r)
        xgt_sb = const.tile([P, 4, G_pad], f32r)
        hgt_sb = const.tile([P, 6, G_pad], f32r)
        og_sb = const.tile([OG, G_pad], f32)

        def emit_graph_phase():
            """Graph head over all chunks: transpose pooled means, then the
            shared-hidden + head matmuls at full G_pad width."""
            for gi, (goff, gsz) in enumerate(GC):
                nc.vector.tensor_copy(out=xg_sb[:gsz, gi, :], in_=xg_ps[gi][:gsz, :])
                for dc in range(4):
                    t_ps2 = ps_t.tile([P, JBLK], f32, tag="t", name="t_ps2")
                    nc.tensor.matmul(
                        t_ps2[:, :gsz],
                        lhsT=xg_sb[:gsz, gi, dc * P : (dc + 1) * P],
                        rhs=ident_r[:gsz, :gsz],
                        start=True,
                        stop=True,
                    )
                    nc.vector.tensor_copy(
                        out=xgt_sb[:, dc, goff : goff + gsz], in_=t_ps2[:, :gsz]
                    )
            for hc in range(6):
                g_ps = ps_hn.tile([P, JBLK], f32, tag="hn", name="g_ps")
                for dc in range(4):
                    nc.tensor.matmul(
                        g_ps[:, :G_pad],
                        lhsT=wg_sb[:, dc, hc * P : (hc + 1) * P],
                        rhs=xgt_sb[:, dc, :],
                        start=(dc == 0),
                        stop=(dc == 3),
                    )
                nc.scalar.activation(
                    out=hgt_sb[:, hc, :], in_=g_ps[:, :G_pad],
                    func=AF.Relu, bias=bg_sb[:, hc : hc + 1],
                )
            og_ps = ps_on.tile([P, JBLK], f32, tag="on", name="og_ps")
            for hc in range(6):
                nc.tensor.matmul(
                    og_ps[:, :G_pad],
                    lhsT=wgh_sb[:, hc, :],
                    rhs=hgt_sb[:, hc, :],
                    start=(hc == 0),
                    stop=(hc == 5),
                )
            nc.scalar.activation(
                out=og_sb[:GHD, :], in_=og_ps[:GHD, :G_pad],
                func=AF.Identity, bias=bgh_sb[:GHD, :],
            )
            nc.scalar.activation(
                out=og_sb[GHD:OG, :], in_=og_ps[GHD:OG, :G_pad],
                func=AF.Square, bias=bgh_sb[GHD:OG, :],
            )
            nc.sync.dma_start(og_out_d[:], og_sb)

        for jb in range(NB):
            # per-a (128-row) loads: the first pooling matmul only needs
            # the a=0 slices, so the PE starts ~4x earlier on each block.
            x_nat = xpool.tile([P, 4, D], f32r, tag="x")
            pt_sb = ptpool.tile([P, 4, G_pad], f32r, tag="pt")
            for a in range(4):
                r0 = jb * JBLK + a * P
                nc.sync.dma_start(x_nat[:, a, :], x_d[r0 : r0 + P, :])
                nc.sync.dma_start(pt_sb[:, a, :], pt_d[r0 : r0 + P, :])

            # graph mean pooling accumulation: xg[g, :] += PT_chunk.T @ x
            for a in range(4):
                for gi, (goff, gsz) in enumerate(GC):
                    bf, bl = pool_act[gi]
                    if not (bf <= jb <= bl):
                        continue
                    nc.tensor.matmul(
                        xg_ps[gi][:gsz, :],
                        lhsT=pt_sb[:, a, goff : goff + gsz],
                        rhs=x_nat[:, a, :],
                        start=(jb == bf and a == 0),
                        stop=(jb == bl and a == 3),
                        skip_group_check=True,
                    )

            # transpose x block: xt[p_dlow, dc, j] = x[node j, dc*128+p]
            xt_sb = xtpool.tile([P, 4, JBLK], f32r, tag="xt")
            for dc in range(4):
                if use_tt:
                    t_ps = ps_t.tile([P, JBLK], f32r, tag="t")
                    for a in range(4):
                        nc.tensor.transpose(
                            t_ps[:, a * P : (a + 1) * P],
                            x_nat[:, a, dc * P : (dc + 1) * P],
                            ident_r,
                        )
                else:
                    t_ps = ps_t.tile([P, JBLK], f32, tag="t")
                    for a in range(4):
                        nc.tensor.matmul(
                            t_ps[:, a * P : (a + 1) * P],
                            lhsT=x_nat[:, a, dc * P : (dc + 1) * P],
                            rhs=ident_r,
                            start=True,
                            stop=True,
                        )
                nc.vector.tensor_copy(out=xt_sb[:, dc, :], in_=t_ps)

            # node MLP layer 1 (transposed): hn_T = relu(Wn1.T-chunks @ x_T + b)
            hn_sb = hnpool.tile([P, 4, JBLK], f32r, tag="hn")
            for d2c in range(4):
                h_ps = ps_hn.tile([P, JBLK], f32, tag="hn")
                for dc in range(4):
                    nc.tensor.matmul(
                        h_ps,
                        lhsT=wn1_sb[:, dc, d2c * P : (d2c + 1) * P],
                        rhs=xt_sb[:, dc, :],
                        start=(dc == 0),
                        stop=(dc == 3),
                    )
                nc.scalar.activation(
                    out=hn_sb[:, d2c, :],
                    in_=h_ps,
                    func=AF.Relu,
                    bias=bn1_sb[:, d2c : d2c + 1],
                )

            # node head: on_T[o, j] = Wn2.T @ hn_T.  Identity and Square
            # copies over all 6 rows; DMA (exempt from the 32-aligned
            # partition-start rule) picks head rows from the identity copy
            # and var rows from the squared copy.
            o_ps = ps_on.tile([P, JBLK], f32, tag="on")
            for d2c in range(4):
                nc.tensor.matmul(
                    o_ps[:ON, :],
                    lhsT=wn2_sb[:, d2c, 0:ON],
                    rhs=hn_sb[:, d2c, :],
                    start=(d2c == 0),
                    stop=(d2c == 3),
                )
            on_id = onpool.tile([ON, JBLK], f32, tag="on_id")
            on_sq = onpool.tile([ON, JBLK], f32, tag="on_sq")
            nc.scalar.activation(
                out=on_id, in_=o_ps[:ON, :], func=AF.Identity, bias=bn2_sb[:ON, :]
            )
            nc.scalar.activation(
                out=on_sq, in_=o_ps[:ON, :], func=AF.Square, bias=bn2_sb[:ON, :]
            )
            nc.sync.dma_start(
                on_out_d[:NHD, jb * JBLK : (jb + 1) * JBLK], on_id[:NHD, :]
            )
            nc.sync.dma_start(
                on_out_d[NHD:ON, jb * JBLK : (jb + 1) * JBLK], on_sq[NHD:ON, :]
            )

            if jb == 0:
                nc.gpsimd.dma_start(wg_sb, wg_d[:].rearrange("(c p) n -> p c n", p=P))
                nc.gpsimd.dma_start(wgh_sb, wgh_d[:].rearrange("(c p) n -> p c n", p=P))
                nc.gpsimd.dma_start(bg_sb, bg_d[:].rearrange("(c p) -> p c", p=P))
                nc.gpsimd.dma_start(bgh_sb, bgh_d[:].unsqueeze(1))

            if jb == NB - 1:
                emit_graph_phase()

        # (graph-head emission happens per-chunk inside the block loop)

    nc.compile()
    _CACHE[key] = nc
    return nc


def _pad_wn2(w):
    out = np.zeros((w.shape[0], 8), np.float32)
    out[:, 0:ON] = w
    return out


def _make_in_maps(x, batch, dataset_name, Wg_shared, bg_shared, Wg_head, bg_head,
                  Wn1, bn1, Wn2, bn2, shards, counts, M_pad, G_pad):
    in_maps = []
    for ci in range(NCORES):
        b = ci // 2
        graphs, node_idx = shards[ci]
        n = len(node_idx)
        g = len(graphs)
        x_sh = np.zeros((M_pad, D), np.float32)
        if n:
            x_sh[:n] = x[node_idx]
        pt = np.zeros((M_pad, G_pad), np.float32)
        if n:
            gcounts = counts[graphs]
            glocal = np.repeat(np.arange(g), gcounts)
            pt[np.arange(n), glocal] = (
                1.0 / np.maximum(gcounts, 1).astype(np.float32)
            )[glocal]
        in_maps.append({
            "x": x_sh,
            "ident": np.eye(P, dtype=np.float32),
            "pt": pt,
            "wn1": np.ascontiguousarray(Wn1[b]),
            "bn1": np.ascontiguousarray(bn1[b]),
            "wn2": _pad_wn2(Wn2[b]),
            "bn2": np.ascontiguousarray(bn2[b]),
            "wg": np.ascontiguousarray(Wg_shared[b]),
            "bg": np.ascontiguousarray(bg_shared[b]),
            "wgh": np.ascontiguousarray(Wg_head[b]),
            "bgh": np.ascontiguousarray(bg_head[b]),
        })
    return in_maps


def _choose_c0(shards, counts, M_pad, G_pad):
    """Pick the chunk-0/chunk-1 graph boundary so chunk 0's nodes end before
    the last node block on every core — lets chunk 0's graph head overlap
    the final node block.  Returns None when a single chunk suffices."""
    if G_pad <= P:
        return None
    NB = M_pad // JBLK
    lim = (NB - 1) * JBLK
    c0 = P
    for graphs, node_idx in shards:
        g = len(graphs)
        if g == 0:
            continue
        cs = np.cumsum(counts[graphs])
        pc = int(np.searchsorted(cs, lim, "right"))
        c0 = min(c0, pc)
    c0 = max(c0, G_pad - P, 1)
    return min(c0, P)


def _pool_activity(shards, counts, M_pad, GC):
    """Per graph-chunk, the (first, last) node-block range (over all cores)
    where that chunk's nodes can appear."""
    act = []
    for goff, gsz in GC:
        bf, bl = None, None
        for graphs, node_idx in shards:
            g = len(graphs)
            if g == 0 or goff >= g:
                continue
            gcounts = counts[graphs]
            glocal = np.repeat(np.arange(g), gcounts)
            jf = int(np.searchsorted(glocal, goff, "left"))
            jl = int(np.searchsorted(glocal, min(goff + gsz, g), "left")) - 1
            if jl < jf:
                continue
            cbf, cbl = jf // JBLK, jl // JBLK
            bf = cbf if bf is None else min(bf, cbf)
            bl = cbl if bl is None else max(bl, cbl)
        if bf is None:
            bf, bl = 0, 0  # chunk empty everywhere: one start-MM writes zeros
        act.append((bf, bl))
    return tuple(act)


def _run(inputs, trace=False, trace_kwargs=None):
    from concourse.bass_utils import run_bass_kernel_spmd

    x = np.asarray(inputs["x"], np.float32)
    batch = np.asarray(inputs["batch"])
    dataset_name = np.asarray(inputs["dataset_name"])
    N = x.shape[0]
    G = dataset_name.shape[0]

    shards, counts = _plan_shards(batch, dataset_name)
    M_pad = max(JBLK, _round_up(max(len(s[1]) for s in shards), JBLK))
    G_pad = max(16, _round_up(max(len(s[0]) for s in shards), 16))
    c0 = None  # single shared graph phase: standard 128-wide chunking
    GC = _g_chunks(G_pad, c0)
    pool_act = _pool_activity(shards, counts, M_pad, GC)

    nc = _build_program(M_pad, G_pad, c0, pool_act)
    in_maps = _make_in_maps(
        x, batch, dataset_name,
        np.asarray(inputs["Wg_shared"], np.float32),
        np.asarray(inputs["bg_shared"], np.float32),
        np.asarray(inputs["Wg_head"], np.float32),
        np.asarray(inputs["bg_head"], np.float32),
        np.asarray(inputs["Wn1"], np.float32),
        np.asarray(inputs["bn1"], np.float32),
        np.asarray(inputs["Wn2"], np.float32),
        np.asarray(inputs["bn2"], np.float32),
        shards, counts, M_pad, G_pad,
    )
    kw = {}
    if trace:
        kw["trace"] = True
        if trace_kwargs:
            kw.update(trace_kwargs)
    res = run_bass_kernel_spmd(nc, in_maps, core_ids=list(range(NCORES)), **kw)

    graph_head = np.zeros((G, GHD), np.float32)
    graph_var = np.zeros((G, GHD), np.float32)
    node_head = np.zeros((N, NHD), np.float32)
    node_var = np.zeros((N, NHD), np.float32)
    for ci in range(NCORES):
        graphs, node_idx = shards[ci]
        g, n = len(graphs), len(node_idx)
        og = res.results[ci]["og_out"]       # [128, G_pad]
        on = res.results[ci]["on_out"]       # [6, M_pad]
        if g:
            graph_head[graphs] = og[:GHD, :g].T
            graph_var[graphs] = og[GHD:OG, :g].T
        if n:
            node_head[node_idx] = on[:NHD, :n].T
            node_var[node_idx] = on[NHD:ON, :n].T
    return (graph_head, node_head, graph_var, node_var), res


def kernel(**inputs):
    outs, _ = _run(inputs, trace=False)
    return outs
